# revision 1
# baseline (speedup 1.0000x reference)
"""Chamfer loss kernel for Trainium2 (8 NeuronCores, data-parallel over batch).

Math: for each batch, d2[m,n] = ||pred_m - gt_n||^2 = p2[m] + g2[n] - 2*dot.
The reference gathers the argmin point and recomputes the distance, which
equals min_n d2[m,n] (resp. min_m), so no argmin/gather is needed:
  fwd_e = sqrt(rowmin(d2) + EPS), bwd_e = sqrt(colmin(d2) + EPS)
  loss = mean(relu(fwd_e - t)) + mean(relu(bwd_e - t))

Device work per core (2 batches): d2 via K=5 fp32 matmul with augmented
operands A = [-2*pred; p2; 1] (lhsT) and B = [gt; 1; g2] (rhs); PE writes
[128,512] PSUM tiles; ACT copies PSUM->SBUF as NEGATED fp16 (s = -d2, so all
reductions are max-based); DVE does the col-max accumulation (elementwise max
across m-tiles) and row-max (binary tree at 2x fp16 rate + final reduce);
GPSIMD partition_all_reduce(max) collapses the col accumulator across
partitions.  Host does the tiny epilogue: negate, sqrt/relu/mean on 128K
values.
"""

import os
from contextlib import ExitStack

import numpy as np

EPS = 1e-8
B, M, N = 16, 4096, 4096
NCORES = 8
B_LOC = B // NCORES  # batches per core

_CACHE = {}


def build_nc(b_loc=B_LOC, m=M, n=N, reps=1, G=4, staggered=False):
    import concourse.bacc as bacc
    import concourse.mybir as mybir
    import concourse.tile as tile
    from concourse import bass_isa
    from concourse.bass import ds

    f32 = mybir.dt.float32
    f16 = mybir.dt.float16
    MAX = mybir.AluOpType.max
    Copy = mybir.ActivationFunctionType.Copy
    E = mybir.EngineType

    nc = bacc.Bacc("TRN2", target_bir_lowering=False, debug=False)
    a_in = nc.dram_tensor("a_in", [b_loc, 5, m], f32, kind="ExternalInput").ap()
    b_in = nc.dram_tensor("b_in", [b_loc, 5, n], f32, kind="ExternalInput").ap()
    n_mt = m // 128
    n_grp = n_mt // G
    # fwd_out[b, p, mt] = max_n(-d2[mt*128+p, n]) = -rowmin
    fwd_out = nc.dram_tensor(
        "fwd_out", [b_loc, 128, n_mt], f32, kind="ExternalOutput"
    ).ap()
    # bwd_out[b, 0, n] = max_m(-d2[m, n]) = -colmin
    bwd_out = nc.dram_tensor(
        "bwd_out", [b_loc, 1, n], f16, kind="ExternalOutput"
    ).ap()

    hints = (E.PE, E.Activation, E.DVE, E.SP, E.Pool)
    with tile.TileContext(nc) as tc, ExitStack() as ctx:
        ab_pool = ctx.enter_context(tc.tile_pool(name="ab", bufs=1))
        ps_pool = ctx.enter_context(tc.tile_pool(name="ps", bufs=2, space="PSUM"))
        sb_pool = ctx.enter_context(tc.tile_pool(name="sb", bufs=2))
        w_pool = ctx.enter_context(tc.tile_pool(name="w", bufs=1))
        cp = ctx.enter_context(tc.tile_pool(name="c", bufs=2))

        for _ in range(reps):
            for b in range(b_loc):
                a_sb = ab_pool.tile([5, m], f32, tag="a")
                b_sb = ab_pool.tile([5, n], f32, tag="b")
                nc.sync.dma_start(out=a_sb, in_=a_in[b])
                nc.sync.dma_start(out=b_sb, in_=b_in[b])

                cacc = cp.tile([128, n], f16, tag="cacc")
                fwd = cp.tile([128, n_mt], f32, tag="fwd")
                wcur = cp.tile([5, G * 128], f32, tag="wcur")
                nc.vector.memset(cacc, -60000.0)

                with tc.For_i(
                    0, n_grp, 1, hint_engines=hints, staggered_reset=staggered
                ) as k:
                    # stage this group's G m-tiles of weights (dynamic src)
                    nc.vector.tensor_copy(
                        out=wcur, in_=a_sb[:, ds(k * (G * 128), G * 128)]
                    )
                    sb = sb_pool.tile([128, G, n], f16, tag="sb")
                    for u in range(G):
                        for h in range(2):
                            ps = ps_pool.tile([128, n // 2], f32, tag="ps")
                            for j in range(n // 2 // 512):
                                n0 = h * (n // 2) + j * 512
                                nc.tensor.matmul(
                                    ps[:, j * 512 : (j + 1) * 512],
                                    wcur[:, u * 128 : (u + 1) * 128],
                                    b_sb[:, n0 : n0 + 512],
                                    start=True,
                                    stop=True,
                                )
                            # negate on the way out of PSUM: sb = -d2 (fp16)
                            nc.scalar.activation(
                                out=sb[:, u, h * (n // 2) : (h + 1) * (n // 2)],
                                in_=ps,
                                func=Copy,
                                scale=-1.0,
                            )
                    # col-max accumulate: fold G m-tiles pairwise into scratch,
                    # then in-place folds, then into cacc
                    t1 = w_pool.tile([128, G // 2, n], f16, tag="t1")
                    nc.vector.tensor_tensor(
                        out=t1, in0=sb[:, 0 : G // 2, :], in1=sb[:, G // 2 : G, :], op=MAX
                    )
                    gg = G // 2
                    while gg > 1:
                        gg //= 2
                        nc.vector.tensor_tensor(
                            out=t1[:, 0:gg, :],
                            in0=t1[:, 0:gg, :],
                            in1=t1[:, gg : 2 * gg, :],
                            op=MAX,
                        )
                    nc.vector.tensor_tensor(
                        out=cacc, in0=cacc, in1=t1[:, 0, :], op=MAX
                    )
                    # row-max: batched binary tree across all G m-tiles,
                    # in place inside the sb slab (colmax already consumed sb)
                    size = n
                    while size > 256:
                        size //= 2
                        nc.vector.tensor_tensor(
                            out=sb[:, :, 0:size],
                            in0=sb[:, :, 0:size],
                            in1=sb[:, :, size : 2 * size],
                            op=MAX,
                        )
                    nc.vector.tensor_reduce(
                        out=fwd[:, ds(k * G, G)],
                        in_=sb[:, :, 0:256],
                        axis=mybir.AxisListType.X,
                        op=MAX,
                    )

                # collapse col accumulator across partitions on GPSIMD
                pr = cp.tile([128, n], f16, tag="pr")
                nc.gpsimd.partition_all_reduce(
                    pr, cacc, channels=128, reduce_op=bass_isa.ReduceOp.max
                )
                nc.sync.dma_start(out=fwd_out[b], in_=fwd)
                nc.sync.dma_start(out=bwd_out[b], in_=pr[0:1, :])
    nc.compile()
    return nc


def _host_prep(predict_pc_6, gt_pc_6):
    """Build augmented matmul operands A (lhsT side) and B (rhs side)."""
    pred = np.ascontiguousarray(predict_pc_6[:, :3, :], dtype=np.float32)
    gt = np.ascontiguousarray(gt_pc_6[:, :3, :], dtype=np.float32)
    A = np.empty((B, 5, M), np.float32)
    A[:, 0:3] = -2.0 * pred
    A[:, 3] = np.einsum("bdm,bdm->bm", pred, pred)
    A[:, 4] = 1.0
    Bm = np.empty((B, 5, N), np.float32)
    Bm[:, 0:3] = gt
    Bm[:, 3] = 1.0
    Bm[:, 4] = np.einsum("bdm,bdm->bm", gt, gt)
    return A, Bm


def kernel(predict_pc_6, gt_pc_6, thresh):
    from concourse.bass_utils import run_bass_kernel_spmd

    predict_pc_6 = np.asarray(predict_pc_6)
    gt_pc_6 = np.asarray(gt_pc_6)
    thresh = np.float32(thresh)

    A, Bm = _host_prep(predict_pc_6, gt_pc_6)

    if "nc" not in _CACHE:
        _CACHE["nc"] = build_nc()
    nc = _CACHE["nc"]

    core_ids = list(range(NCORES))
    in_maps = [
        {
            "a_in": np.ascontiguousarray(A[i * B_LOC : (i + 1) * B_LOC]),
            "b_in": np.ascontiguousarray(Bm[i * B_LOC : (i + 1) * B_LOC]),
        }
        for i in core_ids
    ]
    res = run_bass_kernel_spmd(nc, in_maps, core_ids)
    _CACHE["last_res"] = res

    # Host epilogue on 8 * 2 * (4096 + 4096) values.
    fwd_sum = 0.0
    bwd_sum = 0.0
    for i in core_ids:
        r = res.results[i]
        rowmin = -r["fwd_out"].astype(np.float64).reshape(-1)
        colmin = -r["bwd_out"].astype(np.float64).reshape(-1)
        fwd_e = np.sqrt(rowmin + EPS)
        bwd_e = np.sqrt(colmin + EPS)
        fwd_sum += np.maximum(fwd_e - float(thresh), 0.0).sum()
        bwd_sum += np.maximum(bwd_e - float(thresh), 0.0).sum()

    loss = fwd_sum / (B * M) + bwd_sum / (B * N)
    return np.float32(loss)



# revision 2
# speedup vs baseline: 36.2769x; 36.2769x over previous
"""Chamfer loss kernel for Trainium2 (8 NeuronCores, data-parallel over batch).

Math: for each batch, d2[m,n] = p2[m] + g2[n] - 2*dot(pred_m, gt_n). The
reference's gather+recompute equals min_n d2 (resp. min_m), so only row/col
mins are needed:
  loss = mean(relu(sqrt(rowmin d2 + EPS) - t)) + mean(relu(sqrt(colmin + EPS) - t))

Design (per core = 2 batches):
- The matmul computes -d2 DIRECTLY via an augmented K=20 contraction: row
  blocks of 5 per (direction, batch): A = [2*own; -own2; -1] vs
  B = [other; 1; other2]. Inactive blocks are zero in A, so one weight layout
  covers BOTH chamfer directions (fwd: pred->gt, bwd: gt->pred as a second,
  transposed pass) and both batches — colmin never needs a partition
  reduction, everything is a row-max of -d2.
- One hardware For_i over 128 m-tiles: stage weights [20,128] (ldweights
  can't take register offsets), 8 matmuls (float32r: 1 cycle/col vs 4 for
  plain fp32) into two [128,2048] PSUM tiles (4 banks each, double-buffered),
  and one DVE tensor_reduce(max) per PSUM tile straight from PSUM into
  fwd[:, i].
- reps (for the reps-delta timing harness) run as an OUTER hardware loop, so
  repeated invocations re-execute the same instruction stream the way real
  repeated calls would. Input tiles are double-buffered across reps so the
  next rep's DMA overlaps the current rep's compute.
- Host epilogue: sqrt/relu/mean over 8*2*8192 values (negligible).

float32r matmul numerics cost ~1.2e-3 relative error on the final loss
(tolerance 2e-2); exact-fp32 variant (mm_dtype="f32") measured 4.9e-7.
"""

import numpy as np

EPS = 1e-8
B, M, N = 16, 4096, 4096
NCORES = 8
B_LOC = B // NCORES  # batches per core
K = 20  # contraction: 2 dirs x 2 batches x 5 augmented rows
NT = 128  # m-tiles: 2 dirs x 2 batches x 32

_CACHE = {}


def build_nc(reps=1, mm_dtype="f32r", psum_split=2, reps_mode="loop", rep_dbuf=True):
    import concourse.bacc as bacc
    import concourse.mybir as mybir
    import concourse.tile as tile
    from concourse.bass import ds
    from contextlib import ExitStack

    f32 = mybir.dt.float32
    f32r = mybir.dt.float32r
    MAX = mybir.AluOpType.max
    E = mybir.EngineType

    nc = bacc.Bacc("TRN2", target_bir_lowering=False, debug=False)
    a_in = nc.dram_tensor("a_in", [K, NT * 128], f32, kind="ExternalInput").ap()
    b_in = nc.dram_tensor("b_in", [K, 4096], f32, kind="ExternalInput").ap()
    n_seg = psum_split
    n_iter = NT
    fwd_out = nc.dram_tensor(
        "fwd_out", [128, n_seg, n_iter], f32, kind="ExternalOutput"
    ).ap()

    hints = (E.PE, E.Activation, E.DVE, E.SP, E.Pool)
    seg = 4096 // psum_split
    with tile.TileContext(nc) as tc, ExitStack() as ctx:
        pool = ctx.enter_context(tc.tile_pool(name="sb", bufs=1))
        in_pool = ctx.enter_context(
            tc.tile_pool(name="inp", bufs=2 if rep_dbuf else 1)
        )
        ps_pool = ctx.enter_context(
            tc.tile_pool(name="ps", bufs=psum_split, space="PSUM")
        )
        wp = ctx.enter_context(tc.tile_pool(name="w", bufs=2))

        def rep_body():
            a_sb = in_pool.tile([K, NT * 128], f32, tag="a")
            b_sb = in_pool.tile([K, 4096], f32, tag="b")
            nc.sync.dma_start(out=a_sb, in_=a_in)
            nc.sync.dma_start(out=b_sb, in_=b_in)
            fwd = pool.tile([128, n_seg, n_iter], f32, tag="fwd")
            nc.vector.memset(fwd, 0.0)

            if mm_dtype == "f32r":
                # float32r operands must be produced by a rounding compute op,
                # not a DMA (BIR verifier requirement).
                b_mm = pool.tile([K, 4096], f32r, tag="br")
                nc.vector.tensor_copy(out=b_mm, in_=b_sb)
                w_dt = f32r
            else:
                b_mm = b_sb
                w_dt = f32

            with tc.For_i(0, n_iter, 1, hint_engines=hints) as i:
                wcur = wp.tile([K, 128], w_dt, tag="wc")
                nc.vector.tensor_copy(out=wcur, in_=a_sb[:, ds(i * 128, 128)])
                for h in range(psum_split):
                    ps = ps_pool.tile([128, seg], f32, tag="ps")
                    for j in range(seg // 512):
                        n0 = h * seg + j * 512
                        nc.tensor.matmul(
                            ps[:, j * 512 : (j + 1) * 512],
                            wcur,
                            b_mm[:, n0 : n0 + 512],
                            start=True,
                            stop=True,
                        )
                    nc.vector.tensor_reduce(
                        out=fwd[:, h, ds(i, 1)],
                        in_=ps,
                        axis=mybir.AxisListType.X,
                        op=MAX,
                    )
            nc.sync.dma_start(out=fwd_out, in_=fwd)

        if reps_mode == "loop" and reps > 1:
            with tc.For_i(0, reps, 1, hint_engines=hints):
                rep_body()
        else:
            for _ in range(max(1, reps) if reps_mode != "loop" else 1):
                rep_body()
    nc.compile()
    return nc


def _host_prep(predict_pc_6, gt_pc_6):
    """Build per-core augmented operands A [NCORES, K, NT*128], Bm [NCORES, K, 4096].

    Row blocks (5 rows each): r = d*10 + bb*5 for direction d (0: pred->gt,
    1: gt->pred) and core-local batch bb. A column tile t in 0..127 maps to
    d = t//64, bb = (t//32)%2, m-tile = t%32; only the (d, bb) block rows are
    nonzero there, which also selects the direction/batch on the B side.
    """
    pred = np.ascontiguousarray(predict_pc_6[:, :3, :], dtype=np.float32)
    gt = np.ascontiguousarray(gt_pc_6[:, :3, :], dtype=np.float32)
    p2 = np.einsum("bdm,bdm->bm", pred, pred)
    g2 = np.einsum("bdm,bdm->bm", gt, gt)

    A = np.zeros((NCORES, K, NT * 128), np.float32)
    Bm = np.empty((NCORES, K, 4096), np.float32)
    for c in range(NCORES):
        for bb in range(B_LOC):
            gb = c * B_LOC + bb
            for d in range(2):
                own = pred[gb] if d == 0 else gt[gb]
                own2 = p2[gb] if d == 0 else g2[gb]
                oth = gt[gb] if d == 0 else pred[gb]
                oth2 = g2[gb] if d == 0 else p2[gb]
                r = d * 10 + bb * 5
                Bm[c, r : r + 3] = oth
                Bm[c, r + 3] = 1.0
                Bm[c, r + 4] = oth2
                c0 = (d * 64 + bb * 32) * 128
                A[c, r : r + 3, c0 : c0 + 4096] = 2.0 * own
                A[c, r + 3, c0 : c0 + 4096] = -own2
                A[c, r + 4, c0 : c0 + 4096] = -1.0
    return A, Bm


def _epilogue(results, thresh, n_seg=2):
    """fwd_out[p, s, i] = max over PSUM segment s of -d2 for tile i.
    Tile i: direction = i//64, batch = (i//32)%2, point = (i%32)*128 + p."""
    fwd_sum = 0.0
    bwd_sum = 0.0
    for r in results:
        v = r["fwd_out"].astype(np.float64).reshape(128, n_seg, NT)
        neg = v.max(axis=1)  # [128, NT] rowmax of -d2
        dmin = np.maximum(-neg + EPS, 0.0)
        e = np.sqrt(dmin)
        relu = np.maximum(e - float(thresh), 0.0)
        fwd_sum += relu[:, 0:64].sum()
        bwd_sum += relu[:, 64:128].sum()
    return np.float32(fwd_sum / (B * M) + bwd_sum / (B * N))


def kernel(predict_pc_6, gt_pc_6, thresh):
    from concourse.bass_utils import run_bass_kernel_spmd

    predict_pc_6 = np.asarray(predict_pc_6)
    gt_pc_6 = np.asarray(gt_pc_6)
    thresh = np.float32(thresh)

    A, Bm = _host_prep(predict_pc_6, gt_pc_6)

    if "nc" not in _CACHE:
        _CACHE["nc"] = build_nc()
    nc = _CACHE["nc"]

    core_ids = list(range(NCORES))
    in_maps = [
        {"a_in": np.ascontiguousarray(A[i]), "b_in": np.ascontiguousarray(Bm[i])}
        for i in core_ids
    ]
    res = run_bass_kernel_spmd(nc, in_maps, core_ids)
    _CACHE["last_res"] = res
    return _epilogue([res.results[i] for i in core_ids], thresh)
